# revision 39
# baseline (speedup 1.0000x reference)
"""Trainium2 Bass kernel for ColorAttentionModule (histogram binning + 1x1 convs).

Pipeline per NeuronCore (data-parallel over batch, 2 batches/core):
  layout: 128 partitions = 2 batches x 64 channels, free dim = 65536 pixels.

  Histogram via two-level Gram matmuls on the TensorEngine:
    bin b = 16h + l.  count[16h+l] = sum_n S_h[n]*B_l[n] - sum_n S_{h+1}[n]*B_l[n]
    where S_h = [q >= 16h] (staircase) and B_l selects the low nibble via the
    signed remainder w = q - rne16(q) in [-8, 8] (B_l = [w == c_l], c_l = l or
    l-16; the RNE tie w = +/-8 is absorbed by a constant ones-plane at slot
    l=8 whose count is recovered arithmetically post-gather).
    Indicator planes are built per pixel batch on a TRANSPOSED layout
    (pixels on partitions via DMA xbar transpose): DVE at 4x bf16 plus a few
    planes on ACT (Abs+Relu triangle), S_0 and the ones-plane are persistent
    constants in ping-ponged tiles.  The PE computes all 256 staircase-Gram
    sums as block-diagonal matmuls (8 rows x 16h stationary vs 8 rows x 16l
    moving, single-free-dim contiguous operands), accumulated in PSUM over
    all 512 pixel chunks.  This replaces the O(256 * npix) elementwise
    compare-accumulate scan with O(30 * npix) compares + PE MACs.
    The batch loop is software-pipelined: phase-1 codes + transpose of batch
    b are emitted before the plane/matmul work of batch b-1.

  Phase 1 (fused in the batch loop): exact hist codes q = trunc(rnd(x*256/255))
    via the 2^23 magic-number round + is_gt correction (DVE + ACT), with
    min(y, nextafter(256)) clamping the x*S == 256 rounding edge.
  Phase 3: G psum -> SBUF copy that regroups (l,r') -> (r',l) -> DRAM
    roundtrip gather of the block diagonal -> per-row [128, 256] staircase
    sums -> l=8 ones-column fixup -> staircase difference -> first-index
    argmax (iota-min trick).
  Phase 4: m = [dom <= x < dom+1] via two is_ge, conv1 (negated folded fp16
    weights, att = 1-m absorbed into bias), relu, conv2(+bn2)+sigmoid in
    fp16, out = x * s.
"""
import sys
import numpy as np

sys.path.insert(0, "/opt/trn_rl_repo")

_CACHE = {}

BN_EPS = 1e-5
NBINS = 256
SCALE = float(np.float32(256.0 / 255.0))
MAGIC = float(np.float32(2.0 ** 23))
# largest f32 below 256: clamp y here so q = trunc(y) lands in [0, 255]
# even when x*SCALE rounds up to exactly 256.0 (reference clips to 255).
YCLAMP = float(np.nextafter(np.float32(256.0), np.float32(0.0)))


def _build(hw=65536, kb=8, p4_chunk=2048, dbg=False):
    """Build the Bass module. Returns nc."""
    from contextlib import ExitStack
    import concourse.bass as bass
    import concourse.tile as tile
    from concourse import bacc, mybir

    f32 = mybir.dt.float32
    bf16 = mybir.dt.bfloat16
    f16 = mybir.dt.float16
    Alu = mybir.AluOpType
    Act = mybir.ActivationFunctionType

    P = 128
    bchunk = kb * 128          # pixels per batch
    n_batch = hw // bchunk
    n_p4 = hw // p4_chunk
    n_sub = p4_chunk // 512

    nc = bacc.Bacc(None, target_bir_lowering=False, debug=False)

    x_dram = nc.dram_tensor("x", [P, hw], f32, kind="ExternalInput")
    # phase-4 weights in fp16 (mask/activations are fp16; ~1e-3 rel err)
    w1hi_dram = nc.dram_tensor("w1hi", [128, 128], f16, kind="ExternalInput")
    b1_dram = nc.dram_tensor("b1r", [128, 1], f32, kind="ExternalInput")
    w2hi_dram = nc.dram_tensor("w2hi", [128, 2], f16, kind="ExternalInput")
    b2_dram = nc.dram_tensor("b2r", [2, 1], f32, kind="ExternalInput")
    sel_dram = nc.dram_tensor("sel2", [2, 128], f16, kind="ExternalInput")
    iota_dram = nc.dram_tensor("iota", [128, NBINS], f32, kind="ExternalInput")
    out_dram = nc.dram_tensor("out", [P, hw], f32, kind="ExternalOutput")
    # DRAM scratch for the block-diagonal gather (G blocks -> per-row counts)
    gd_dram = nc.dram_tensor("gd", [P, 16 * 128], f32, kind="Internal")
    if dbg:
        cnt_dram = nc.dram_tensor("cnt_dbg", [P, NBINS], f32, kind="ExternalOutput")
        dom_dram = nc.dram_tensor("dom_dbg", [P, 1], f32, kind="ExternalOutput")

    with tile.TileContext(nc) as tc, ExitStack() as top:
        const = top.enter_context(tc.tile_pool(name="const", bufs=1))

        w1hit = const.tile([128, 128], f16)
        nc.sync.dma_start(w1hit[:], w1hi_dram.ap())
        b1t = const.tile([128, 1], f32)
        nc.sync.dma_start(b1t[:], b1_dram.ap())
        w2hit = const.tile([128, 2], f16)
        nc.sync.dma_start(w2hit[:], w2hi_dram.ap())
        b2t = const.tile([2, 1], f32)
        nc.sync.dma_start(b2t[:], b2_dram.ap())
        selt = const.tile([2, 128], f16)
        nc.sync.dma_start(selt[:], sel_dram.ap())
        iotat = const.tile([128, NBINS], f32)
        nc.sync.dma_start(iotat[:], iota_dram.ap())

        # per-partition bias columns for the ACT Abs-based B-planes:
        # column l holds -c_l where c_l = l (l<8) or l-16 (l>8)
        negb = const.tile([128, 16], f32)
        for l in range(16):
            cval = l if l < 8 else l - 16
            nc.gpsimd.memset(negb[:, l:l + 1], float(-cval))

        sc = const.tile([128, NBINS], f32)      # gathered staircase sums
        dcnt = const.tile([128, NBINS], f32)    # per-bin counts
        mx = const.tile([128, 1], f32)
        dom = const.tile([128, 1], f32)
        domp1 = const.tile([128, 1], f32)

        # ---- Phases 1+2: codes, transpose, indicators, Gram matmuls ----
        with ExitStack() as mid:
            gpsum = mid.enter_context(
                tc.tile_pool(name="gpsum", bufs=1, space="PSUM"))
            gbank = [gpsum.tile([128, 512], f32, tag=f"g{j}", name=f"g{j}")
                     for j in range(4)]

            px = mid.enter_context(tc.tile_pool(name="px", bufs=2))
            p1s = mid.enter_context(tc.tile_pool(name="p1s", bufs=2))
            qn_p = mid.enter_context(tc.tile_pool(name="qn_p", bufs=2))
            qt_p = mid.enter_context(tc.tile_pool(name="qt_p", bufs=3))
            nib = mid.enter_context(tc.tile_pool(name="nib", bufs=2))
            ab = mid.enter_context(tc.tile_pool(name="ab", bufs=1))

            # Explicit ping-pong indicator tiles so constant planes persist:
            # A h=0 slot is S_0 = [q >= 0] = 1 (exact); B l=15 slot is ones,
            # making Gram cell (h, 15) = T_h = sum S_h, fixed up after the
            # gather: SC[h,15] := T_h - sum_{l<15} SC[h,l].
            abtiles = []
            for pp in range(2):
                a_t = ab.tile([P, kb, 16, 16, 8], bf16, tag=f"a{pp}",
                              name=f"a{pp}")
                b_t = ab.tile([P, kb, 16, 16, 8], bf16, tag=f"b{pp}",
                              name=f"b{pp}")
                nc.vector.memset(a_t[:, :, :, 0, :], 1.0)
                nc.vector.memset(b_t[:, :, :, 8, :], 1.0)
                abtiles.append((a_t, b_t))

            # Software-pipelined: emit phase-1 (codes + transpose) for batch b
            # BEFORE the nibble/plane/matmul work of batch b-1, so the DVE
            # never stalls on the in-flight DMA transpose + ACT magic ops.
            ACT_B = 3  # B-planes built on ACT via relu(1 - |ql - l|)
            qt_prev = None
            for b in range(n_batch + 1):
                if b < n_batch:
                    sl = slice(b * bchunk, (b + 1) * bchunk)
                    xt = px.tile([P, bchunk], f32, tag="xt")
                    nc.sync.dma_start(xt[:], x_dram.ap()[:, sl])
                    # q = trunc(rnd(x*S)): y = x*S; f = rne(y) via 2^23 magic;
                    # q = f - (f > y).  (f > y) reuses xt (dead after y).
                    yt = p1s.tile([P, bchunk], f32, tag="yt")
                    nc.vector.tensor_scalar(out=yt[:], in0=xt[:], scalar1=SCALE,
                                            scalar2=YCLAMP, op0=Alu.mult,
                                            op1=Alu.min)
                    mt = p1s.tile([P, bchunk], f32, tag="mt")
                    nc.scalar.activation(out=mt[:], in_=yt[:], func=Act.Copy,
                                         bias=MAGIC, scale=1.0)
                    ft = p1s.tile([P, bchunk], f32, tag="ft")
                    nc.scalar.activation(out=ft[:], in_=mt[:], func=Act.Copy,
                                         bias=-MAGIC, scale=1.0)
                    nc.vector.tensor_tensor(out=xt[:], in0=ft[:], in1=yt[:],
                                            op=Alu.is_gt)
                    qn = qn_p.tile([P, bchunk], bf16, tag="qn")
                    nc.vector.tensor_tensor(out=qn[:], in0=ft[:], in1=xt[:],
                                            op=Alu.subtract)
                    # transpose: qt[pp, t, r] = qn[r, t*128+pp]
                    qt = qt_p.tile([P, kb, 128], bf16, tag="qt")
                    nc.sync.dma_start_transpose(qt[:], qn[:])
                    qt_cur = qt
                else:
                    qt_cur = None

                if qt_prev is None:
                    qt_prev = qt_cur
                    continue
                bm = b - 1
                qt = qt_prev
                qt_prev = qt_cur
                qtf = qt[:].rearrange("p t r -> p (t r)")
                # signed remainder: w = q - rne16(q) in [-8, 8] (2048 magic;
                # bf16 ulp there is 16).  w == (q%16) for q%16 < 8,
                # (q%16)-16 for q%16 > 8, and +/-8 (RNE parity) for q%16==8.
                # B-planes compare w directly; the q%16==8 bin is recovered
                # post-gather from the ones-plane column (slot l=8).
                ut = nib.tile([P, kb * 128], bf16, tag="ut")
                nc.scalar.activation(out=ut[:], in_=qtf, func=Act.Copy,
                                     bias=2048.0, scale=1.0)
                vt = nib.tile([P, kb * 128], bf16, tag="vt")
                nc.scalar.activation(out=vt[:], in_=ut[:], func=Act.Copy,
                                     bias=-2048.0, scale=1.0)
                ql = nib.tile([P, kb, 128], bf16, tag="ql")
                qlf = ql[:].rearrange("p t r -> p (t r)")
                nc.vector.tensor_tensor(out=qlf, in0=qtf, in1=vt[:],
                                        op=Alu.subtract)
                # indicator planes, layout [p, t, g, h|l, r]: the (h, r) block
                # for one (t, g) is 128 contiguous elems -> single-free-dim
                # matmul operand; plane writes are 8-elem-contiguous runs.
                # h=0 (A) and l=15 (B) slots hold persistent constant ones.
                at_, bt_ = abtiles[bm % 2]
                at_flat = at_[:].rearrange("p t g h r -> p (t g h r)")
                bt_flat = bt_[:].rearrange("p t g h r -> p (t g h r)")
                qtv = qt[:].rearrange("p t (g r) -> p t g r", g=16)
                qlv = ql[:].rearrange("p t (g r) -> p t g r", g=16)
                ab1 = nib.tile([P, kb * 128], bf16, tag="ab1")
                for h in range(1, 16):
                    nc.vector.tensor_scalar(
                        out=at_[:, :, :, h, :], in0=qtv,
                        scalar1=float(16 * h), scalar2=None, op0=Alu.is_ge)
                bslots = list(range(8)) + list(range(9, 16))
                for i, l in enumerate(bslots):
                    cval = float(l if l < 8 else l - 16)
                    if i < 15 - ACT_B:
                        nc.vector.tensor_scalar(
                            out=bt_[:, :, :, l, :], in0=qlv,
                            scalar1=cval, scalar2=None, op0=Alu.is_equal)
                    else:
                        # ACT path: B_l = relu(1 - |w - c|), exact on ints
                        nc.scalar.activation(out=ab1[:], in_=qlf,
                                             func=Act.Abs,
                                             bias=negb[:, l:l + 1],
                                             scale=1.0)
                        nc.scalar.activation(
                            out=bt_[:, :, :, l, :],
                            in_=ab1[:].rearrange("p (t g r) -> p t g r",
                                                 t=kb, g=16),
                            func=Act.Relu, bias=1.0, scale=-1.0)
                # Gram matmuls: G_g[(h,r),(l,r')] += sum_pix S_h*B_l
                for t in range(kb):
                    for g in range(16):
                        c0 = (t * 16 + g) * 128
                        lhsT = at_flat[:, c0:c0 + 128]
                        rhs = bt_flat[:, c0:c0 + 128]
                        gg = gbank[g // 4][:, (g % 4) * 128:(g % 4 + 1) * 128]
                        # start=True lazily zeroes the WHOLE 2KB psum bank
                        # (zero region), so only the first group per bank may
                        # set it; groups 1-3 of the bank inherit pending-zero.
                        nc.tensor.matmul(
                            gg, lhsT, rhs,
                            start=(bm == 0 and t == 0 and g % 4 == 0),
                            stop=(bm == n_batch - 1 and t == kb - 1
                                  and g % 4 == 3))

            # ---- Phase 3a: stage G to SBUF, roundtrip via DRAM to gather ----
            # The psum->sbuf copy also regroups moving cols (l, r') -> (r', l)
            # so the per-row l-runs are contiguous for the gather DMA.
            gs = const.tile([128, 16 * 128], f32)
            for j in range(4):
                dstv = gs[:, j * 512:(j + 1) * 512].rearrange(
                    "p (g r l) -> p g r l", g=4, r=8)
                srcv = gbank[j][:].rearrange(
                    "p (g l r) -> p g r l", g=4, l=16)
                nc.scalar.activation(out=dstv, in_=srcv, func=Act.Copy,
                                     bias=0.0, scale=1.0)
            nc.sync.dma_start(gd_dram.ap(), gs[:])
            # gather: sc[g*8+r, 16h+l] = gd[8h+r, g*128 + 16r + l]
            # One DMA per g: dst [8 rows, 16h, 16l]; src diagonal AP with
            # r-stride = 2048 + 16 (flat DRAM elements), contiguous l runs.
            for g in range(16):
                dst = sc[g * 8:(g + 1) * 8, :].rearrange(
                    "p (h l) -> p h l", h=16)
                src = bass.AP(gd_dram, g * 128,
                              [[2048 + 16, 8], [8 * 2048, 16], [1, 16]])
                nc.sync.dma_start(dst, src)

        # ---- Phase 3b: staircase difference + first-index argmax ----
        # count[16h+l] = sc[16h+l] - sc[16(h+1)+l]
        nc.vector.tensor_tensor(out=dcnt[:, 0:240], in0=sc[:, 0:240],
                                in1=sc[:, 16:256], op=Alu.subtract)
        nc.vector.tensor_copy(dcnt[:, 240:256], sc[:, 240:256])
        # B l=8 slot was constant ones, so col 8 of dcnt currently holds
        # T_h - T_{h+1} (all q in [16h,16h+16)).  True count[16h+8]
        # = (T_h - T_{h+1}) - sum_{l != 8} count[16h+l] = 2*raw - rowsum.
        dc3 = dcnt[:].rearrange("p (h l) -> p h l", h=16)
        sall = const.tile([128, 16], f32)
        nc.vector.tensor_reduce(out=sall[:], in_=dc3,
                                axis=mybir.AxisListType.X, op=Alu.add)
        t16 = const.tile([128, 16, 1], f32)
        nc.vector.tensor_scalar(out=t16[:], in0=dc3[:, :, 8:9], scalar1=2.0,
                                scalar2=None, op0=Alu.mult)
        nc.vector.tensor_tensor(out=dc3[:, :, 8:9], in0=t16[:],
                                in1=sall[:].rearrange("p h -> p h ()"),
                                op=Alu.subtract)
        nc.vector.tensor_reduce(out=mx[:], in_=dcnt[:],
                                axis=mybir.AxisListType.X, op=Alu.max)
        t1 = const.tile([128, NBINS], f32)
        nc.vector.tensor_scalar(out=t1[:], in0=dcnt[:], scalar1=mx[:],
                                scalar2=1.0e6, op0=Alu.not_equal, op1=Alu.mult)
        nc.vector.tensor_tensor(out=t1[:], in0=t1[:], in1=iotat[:], op=Alu.add)
        nc.vector.tensor_reduce(out=dom[:], in_=t1[:],
                                axis=mybir.AxisListType.X, op=Alu.min)
        nc.vector.tensor_scalar(out=domp1[:], in0=dom[:], scalar1=1.0,
                                scalar2=None, op0=Alu.add)
        if dbg:
            nc.sync.dma_start(cnt_dram.ap(), dcnt[:])
            nc.sync.dma_start(dom_dram.ap(), dom[:])

        # ---- Phase 4: mask, convs, output ----
        # m = [dom <= x < dom+1]; att = 1 - m folded into (negated W1, b1)
        with ExitStack() as p4:
            px4 = p4.enter_context(tc.tile_pool(name="px4", bufs=3))
            pw = p4.enter_context(tc.tile_pool(name="pw", bufs=2))
            pz = p4.enter_context(tc.tile_pool(name="pz", bufs=2))
            pout = p4.enter_context(tc.tile_pool(name="pout", bufs=3))
            ps_z = p4.enter_context(tc.tile_pool(name="ps_z", bufs=3, space="PSUM"))
            ps_s = p4.enter_context(tc.tile_pool(name="ps_s", bufs=2, space="PSUM"))
            ps_b = p4.enter_context(tc.tile_pool(name="ps_b", bufs=3, space="PSUM"))

            for j in range(n_p4):
                sl = slice(j * p4_chunk, (j + 1) * p4_chunk)
                xt = px4.tile([P, p4_chunk], f32, tag="xt")
                nc.sync.dma_start(xt[:], x_dram.ap()[:, sl])
                ga = pw.tile([P, p4_chunk], f16, tag="ga")
                nc.vector.tensor_scalar(out=ga[:], in0=xt[:], scalar1=dom[:],
                                        scalar2=None, op0=Alu.is_ge)
                gb = pw.tile([P, p4_chunk], f16, tag="gb")
                nc.vector.tensor_scalar(out=gb[:], in0=xt[:], scalar1=domp1[:],
                                        scalar2=None, op0=Alu.is_ge)
                mt = pw.tile([P, p4_chunk], f16, tag="mt")
                nc.vector.tensor_tensor(out=mt[:], in0=ga[:], in1=gb[:],
                                        op=Alu.subtract)
                zt = pz.tile([P, p4_chunk], f16, tag="zt")
                st = pz.tile([2, p4_chunk], f16, tag="st")
                ot = pout.tile([P, p4_chunk], f32, tag="ot")
                for k in range(n_sub):
                    ssl = slice(k * 512, (k + 1) * 512)
                    zp = ps_z.tile([128, 512], f32, tag="zp")
                    nc.tensor.matmul(zp[:], w1hit[:], mt[:, ssl],
                                     start=True, stop=True)
                    nc.scalar.activation(out=zt[:, ssl], in_=zp[:], func=Act.Relu,
                                         bias=b1t[:], scale=1.0)
                    sp = ps_s.tile([2, 512], f32, tag="sp")
                    nc.tensor.matmul(sp[:], w2hit[:], zt[:, ssl],
                                     start=True, stop=True)
                    nc.scalar.activation(out=st[:, ssl], in_=sp[:], func=Act.Sigmoid,
                                         bias=b2t[:], scale=1.0)
                    bp = ps_b.tile([128, 512], f32, tag="bp")
                    nc.tensor.matmul(bp[:], selt[:], st[:, ssl], start=True, stop=True)
                    nc.vector.tensor_tensor(out=ot[:, ssl], in0=xt[:, ssl], in1=bp[:],
                                            op=Alu.mult)
                nc.sync.dma_start(out_dram.ap()[:, sl], ot[:])

    if not nc.is_finalized():
        nc.finalize()
    return nc


def _host_constants(conv1_w, conv1_b, bn1_gamma, bn1_beta, bn1_mean, bn1_var,
                    conv2_w, conv2_b, bn2_gamma, bn2_beta, bn2_mean, bn2_var):
    """Fold BN into conv weights (float64, cast f32) and build layout blocks.

    Phase 4 computes m = [dom <= x < dom+1] = 1 - att, so conv1 is applied with
    negated weights and bias shifted by the row sums: W1'(1-m) = (W1'*1 - W1'*m).
    """
    C = conv1_w.shape[0]
    inv1 = (bn1_gamma.astype(np.float64)
            / np.sqrt(bn1_var.astype(np.float64) + BN_EPS))
    w1f = conv1_w.astype(np.float64) * inv1[:, None]          # [o, c]
    b1f = (conv1_b.astype(np.float64) * inv1
           + bn1_beta.astype(np.float64)
           - bn1_mean.astype(np.float64) * inv1)              # [o]
    # att = 1 - m fold
    b1n = b1f + w1f.sum(axis=1)
    w1n = -w1f

    inv2 = (bn2_gamma.astype(np.float64)
            / np.sqrt(bn2_var.astype(np.float64) + BN_EPS))
    w2f = conv2_w[0].astype(np.float64) * inv2[0]             # [c]
    b2f = (conv2_b.astype(np.float64) * inv2
           + bn2_beta.astype(np.float64)
           - bn2_mean.astype(np.float64) * inv2)              # [1]

    fp16 = np.float16

    w1blk = np.zeros((128, 128), np.float32)
    w1t = w1n.T.astype(np.float32)                            # [c, o]
    w1blk[:C, :C] = w1t
    w1blk[C:, C:] = w1t
    w1hi = w1blk.astype(fp16)
    b1r = np.tile(b1n.astype(np.float32), 2).reshape(128, 1)

    w2blk = np.zeros((128, 2), np.float32)
    w2blk[:C, 0] = w2f.astype(np.float32)
    w2blk[C:, 1] = w2f.astype(np.float32)
    w2hi = w2blk.astype(fp16)
    b2r = np.full((2, 1), b2f[0], np.float32)

    sel2 = np.zeros((2, 128), fp16)
    sel2[0, :C] = 1.0
    sel2[1, C:] = 1.0

    iota = np.tile(np.arange(NBINS, dtype=np.float32), (128, 1))
    return dict(w1hi=w1hi, b1r=b1r, w2hi=w2hi, b2r=b2r, sel2=sel2, iota=iota)


def _run(x, conv1_w, conv1_b, bn1_gamma, bn1_beta, bn1_mean, bn1_var,
         conv2_w, conv2_b, bn2_gamma, bn2_beta, bn2_mean, bn2_var,
         trace=False):
    from concourse.bass_utils import run_bass_kernel_spmd

    x = np.asarray(x, np.float32)
    B, C, H, W = x.shape
    hw = H * W
    n_cores = 8
    bpc = B // n_cores  # batches per core

    key = ("nc", hw)
    if key not in _CACHE:
        _CACHE[key] = _build(hw=hw)
    nc = _CACHE[key]

    consts = _host_constants(
        np.asarray(conv1_w), np.asarray(conv1_b), np.asarray(bn1_gamma),
        np.asarray(bn1_beta), np.asarray(bn1_mean), np.asarray(bn1_var),
        np.asarray(conv2_w), np.asarray(conv2_b), np.asarray(bn2_gamma),
        np.asarray(bn2_beta), np.asarray(bn2_mean), np.asarray(bn2_var))

    xs = x.reshape(n_cores, bpc * C, hw)
    in_maps = [dict(x=np.ascontiguousarray(xs[i]), **consts) for i in range(n_cores)]

    res = run_bass_kernel_spmd(nc, in_maps, core_ids=list(range(n_cores)),
                               trace=trace)
    outs = [res.results[i]["out"].reshape(bpc, C, H, W) for i in range(n_cores)]
    return np.concatenate(outs, axis=0).astype(np.float32), res


def kernel(**inputs):
    out, _ = _run(**inputs)
    return out


# revision 41
# speedup vs baseline: 1.0116x; 1.0116x over previous
"""Trainium2 Bass kernel for ColorAttentionModule (histogram binning + 1x1 convs).

Pipeline per NeuronCore (data-parallel over batch, 2 batches/core):
  layout: 128 partitions = 2 batches x 64 channels, free dim = 65536 pixels.

  Histogram via two-level Gram matmuls on the TensorEngine:
    bin b = 16h + l.  count[16h+l] = sum_n S_h[n]*B_l[n] - sum_n S_{h+1}[n]*B_l[n]
    where S_h = [q >= 16h] (staircase) and B_l selects the low nibble via the
    signed remainder w = q - rne16(q) in [-8, 8] (B_l = [w == c_l], c_l = l or
    l-16; the RNE tie w = +/-8 is absorbed by a constant ones-plane at slot
    l=8 whose count is recovered arithmetically post-gather).
    Indicator planes are built per pixel batch on a TRANSPOSED layout
    (pixels on partitions via DMA xbar transpose): DVE at 4x bf16 plus a few
    planes on ACT (Abs+Relu triangle), S_0 and the ones-plane are persistent
    constants in ping-ponged tiles.  The PE computes all 256 staircase-Gram
    sums as block-diagonal matmuls (8 rows x 16h stationary vs 8 rows x 16l
    moving, single-free-dim contiguous operands), accumulated in PSUM over
    all 512 pixel chunks.  This replaces the O(256 * npix) elementwise
    compare-accumulate scan with O(30 * npix) compares + PE MACs.
    The batch loop is software-pipelined: phase-1 codes + transpose of batch
    b are emitted before the plane/matmul work of batch b-1.

  Phase 1 (fused in the batch loop): exact hist codes q = trunc(rnd(x*256/255))
    via the 2^23 magic-number round + is_gt correction (DVE + ACT), with
    min(y, nextafter(256)) clamping the x*S == 256 rounding edge.
  Phase 3: G psum -> SBUF copy that regroups (l,r') -> (r',l) -> DRAM
    roundtrip gather of the block diagonal -> per-row [128, 256] staircase
    sums -> l=8 ones-column fixup -> staircase difference -> first-index
    argmax (iota-min trick).
  Phase 4: m = [dom <= x < dom+1] via two is_ge, conv1 (negated folded fp16
    weights, att = 1-m absorbed into bias), relu, conv2(+bn2)+sigmoid in
    fp16, out = x * s.
"""
import sys
import numpy as np

sys.path.insert(0, "/opt/trn_rl_repo")

_CACHE = {}

BN_EPS = 1e-5
NBINS = 256
SCALE = float(np.float32(256.0 / 255.0))
MAGIC = float(np.float32(2.0 ** 23))
# largest f32 below 256: clamp y here so q = trunc(y) lands in [0, 255]
# even when x*SCALE rounds up to exactly 256.0 (reference clips to 255).
YCLAMP = float(np.nextafter(np.float32(256.0), np.float32(0.0)))


def _build(hw=65536, kb=8, p4_chunk=2048, dbg=False):
    """Build the Bass module. Returns nc."""
    from contextlib import ExitStack
    import concourse.bass as bass
    import concourse.tile as tile
    from concourse import bacc, mybir

    f32 = mybir.dt.float32
    bf16 = mybir.dt.bfloat16
    f16 = mybir.dt.float16
    Alu = mybir.AluOpType
    Act = mybir.ActivationFunctionType

    P = 128
    bchunk = kb * 128          # pixels per batch
    n_batch = hw // bchunk
    n_p4 = hw // p4_chunk
    n_sub = p4_chunk // 512

    nc = bacc.Bacc(None, target_bir_lowering=False, debug=False)

    x_dram = nc.dram_tensor("x", [P, hw], f32, kind="ExternalInput")
    # phase-4 weights in fp16 (mask/activations are fp16; ~1e-3 rel err)
    w1hi_dram = nc.dram_tensor("w1hi", [128, 128], f16, kind="ExternalInput")
    b1_dram = nc.dram_tensor("b1r", [128, 1], f32, kind="ExternalInput")
    w2hi_dram = nc.dram_tensor("w2hi", [128, 2], f16, kind="ExternalInput")
    b2_dram = nc.dram_tensor("b2r", [2, 1], f32, kind="ExternalInput")
    sel_dram = nc.dram_tensor("sel2", [2, 128], f16, kind="ExternalInput")
    iota_dram = nc.dram_tensor("iota", [128, NBINS], f32, kind="ExternalInput")
    out_dram = nc.dram_tensor("out", [P, hw], f32, kind="ExternalOutput")
    # DRAM scratch for the block-diagonal gather (G blocks -> per-row counts)
    gd_dram = nc.dram_tensor("gd", [P, 16 * 128], f32, kind="Internal")
    if dbg:
        cnt_dram = nc.dram_tensor("cnt_dbg", [P, NBINS], f32, kind="ExternalOutput")
        dom_dram = nc.dram_tensor("dom_dbg", [P, 1], f32, kind="ExternalOutput")

    with tile.TileContext(nc) as tc, ExitStack() as top:
        const = top.enter_context(tc.tile_pool(name="const", bufs=1))

        w1hit = const.tile([128, 128], f16)
        nc.sync.dma_start(w1hit[:], w1hi_dram.ap())
        b1t = const.tile([128, 1], f32)
        nc.sync.dma_start(b1t[:], b1_dram.ap())
        w2hit = const.tile([128, 2], f16)
        nc.sync.dma_start(w2hit[:], w2hi_dram.ap())
        b2t = const.tile([2, 1], f32)
        nc.sync.dma_start(b2t[:], b2_dram.ap())
        selt = const.tile([2, 128], f16)
        nc.sync.dma_start(selt[:], sel_dram.ap())
        iotat = const.tile([128, NBINS], f32)
        nc.sync.dma_start(iotat[:], iota_dram.ap())

        # per-partition bias columns for the ACT Abs-based B-planes:
        # column l holds -c_l where c_l = l (l<8) or l-16 (l>8)
        negb = const.tile([128, 16], f32)
        for l in range(16):
            cval = l if l < 8 else l - 16
            nc.gpsimd.memset(negb[:, l:l + 1], float(-cval))

        sc = const.tile([128, NBINS], f32)      # gathered staircase sums
        dcnt = const.tile([128, NBINS], f32)    # per-bin counts
        mx = const.tile([128, 1], f32)
        dom = const.tile([128, 1], f32)
        domp1 = const.tile([128, 1], f32)

        # ---- Phases 1+2: codes, transpose, indicators, Gram matmuls ----
        with ExitStack() as mid:
            gpsum = mid.enter_context(
                tc.tile_pool(name="gpsum", bufs=1, space="PSUM"))
            gbank = [gpsum.tile([128, 512], f32, tag=f"g{j}", name=f"g{j}")
                     for j in range(4)]

            px = mid.enter_context(tc.tile_pool(name="px", bufs=2))
            p1s = mid.enter_context(tc.tile_pool(name="p1s", bufs=2))
            qn_p = mid.enter_context(tc.tile_pool(name="qn_p", bufs=2))
            qt_p = mid.enter_context(tc.tile_pool(name="qt_p", bufs=3))
            nib = mid.enter_context(tc.tile_pool(name="nib", bufs=2))
            ab = mid.enter_context(tc.tile_pool(name="ab", bufs=1))

            # Explicit ping-pong indicator tiles so constant planes persist:
            # A h=0 slot is S_0 = [q >= 0] = 1 (exact); B l=15 slot is ones,
            # making Gram cell (h, 15) = T_h = sum S_h, fixed up after the
            # gather: SC[h,15] := T_h - sum_{l<15} SC[h,l].
            abtiles = []
            for pp in range(2):
                a_t = ab.tile([P, kb, 16, 16, 8], bf16, tag=f"a{pp}",
                              name=f"a{pp}")
                b_t = ab.tile([P, kb, 16, 16, 8], bf16, tag=f"b{pp}",
                              name=f"b{pp}")
                nc.vector.memset(a_t[:, :, :, 0, :], 1.0)
                nc.vector.memset(b_t[:, :, :, 8, :], 1.0)
                abtiles.append((a_t, b_t))

            # Software-pipelined: emit phase-1 (codes + transpose) for batch b
            # BEFORE the nibble/plane/matmul work of batch b-1, so the DVE
            # never stalls on the in-flight DMA transpose + ACT magic ops.
            ACT_B = 3  # B-planes built on ACT via relu(1 - |ql - l|)
            qt_prev = None
            for b in range(n_batch + 1):
                if b < n_batch:
                    sl = slice(b * bchunk, (b + 1) * bchunk)
                    xt = px.tile([P, bchunk], f32, tag="xt")
                    nc.sync.dma_start(xt[:], x_dram.ap()[:, sl])
                    # q = trunc(rnd(x*S)): y = x*S; f = rne(y) via 2^23 magic;
                    # q = f - (f > y).  (f > y) reuses xt (dead after y).
                    yt = p1s.tile([P, bchunk], f32, tag="yt")
                    nc.vector.tensor_scalar(out=yt[:], in0=xt[:], scalar1=SCALE,
                                            scalar2=YCLAMP, op0=Alu.mult,
                                            op1=Alu.min)
                    mt = p1s.tile([P, bchunk], f32, tag="mt")
                    nc.scalar.activation(out=mt[:], in_=yt[:], func=Act.Copy,
                                         bias=MAGIC, scale=1.0)
                    ft = p1s.tile([P, bchunk], f32, tag="ft")
                    nc.scalar.activation(out=ft[:], in_=mt[:], func=Act.Copy,
                                         bias=-MAGIC, scale=1.0)
                    nc.vector.tensor_tensor(out=xt[:], in0=ft[:], in1=yt[:],
                                            op=Alu.is_gt)
                    qn = qn_p.tile([P, bchunk], bf16, tag="qn")
                    nc.vector.tensor_tensor(out=qn[:], in0=ft[:], in1=xt[:],
                                            op=Alu.subtract)
                    # transpose: qt[pp, t, r] = qn[r, t*128+pp]
                    qt = qt_p.tile([P, kb, 128], bf16, tag="qt")
                    nc.sync.dma_start_transpose(qt[:], qn[:])
                    qt_cur = qt
                else:
                    qt_cur = None

                if qt_prev is None:
                    qt_prev = qt_cur
                    continue
                bm = b - 1
                qt = qt_prev
                qt_prev = qt_cur
                qtf = qt[:].rearrange("p t r -> p (t r)")
                # signed remainder: w = q - rne16(q) in [-8, 8] (2048 magic;
                # bf16 ulp there is 16).  w == (q%16) for q%16 < 8,
                # (q%16)-16 for q%16 > 8, and +/-8 (RNE parity) for q%16==8.
                # B-planes compare w directly; the q%16==8 bin is recovered
                # post-gather from the ones-plane column (slot l=8).
                ut = nib.tile([P, kb * 128], bf16, tag="ut")
                nc.scalar.activation(out=ut[:], in_=qtf, func=Act.Copy,
                                     bias=2048.0, scale=1.0)
                vt = nib.tile([P, kb * 128], bf16, tag="vt")
                nc.scalar.activation(out=vt[:], in_=ut[:], func=Act.Copy,
                                     bias=-2048.0, scale=1.0)
                ql = nib.tile([P, kb, 128], bf16, tag="ql")
                qlf = ql[:].rearrange("p t r -> p (t r)")
                nc.vector.tensor_tensor(out=qlf, in0=qtf, in1=vt[:],
                                        op=Alu.subtract)
                # indicator planes, layout [p, t, g, h|l, r]: the (h, r) block
                # for one (t, g) is 128 contiguous elems -> single-free-dim
                # matmul operand; plane writes are 8-elem-contiguous runs.
                # h=0 (A) and l=15 (B) slots hold persistent constant ones.
                at_, bt_ = abtiles[bm % 2]
                at_flat = at_[:].rearrange("p t g h r -> p (t g h r)")
                bt_flat = bt_[:].rearrange("p t g h r -> p (t g h r)")
                qtv = qt[:].rearrange("p t (g r) -> p t g r", g=16)
                qlv = ql[:].rearrange("p t (g r) -> p t g r", g=16)
                ab1 = nib.tile([P, kb * 128], bf16, tag="ab1")
                for h in range(1, 16):
                    nc.vector.tensor_scalar(
                        out=at_[:, :, :, h, :], in0=qtv,
                        scalar1=float(16 * h), scalar2=None, op0=Alu.is_ge)
                bslots = list(range(8)) + list(range(9, 16))
                for i, l in enumerate(bslots):
                    cval = float(l if l < 8 else l - 16)
                    if i < 15 - ACT_B:
                        nc.vector.tensor_scalar(
                            out=bt_[:, :, :, l, :], in0=qlv,
                            scalar1=cval, scalar2=None, op0=Alu.is_equal)
                    else:
                        # ACT path: B_l = relu(1 - |w - c|), exact on ints
                        nc.scalar.activation(out=ab1[:], in_=qlf,
                                             func=Act.Abs,
                                             bias=negb[:, l:l + 1],
                                             scale=1.0)
                        nc.scalar.activation(
                            out=bt_[:, :, :, l, :],
                            in_=ab1[:].rearrange("p (t g r) -> p t g r",
                                                 t=kb, g=16),
                            func=Act.Relu, bias=1.0, scale=-1.0)
                # Gram matmuls: G_g[(h,r),(l,r')] += sum_pix S_h*B_l
                for t in range(kb):
                    for g in range(16):
                        c0 = (t * 16 + g) * 128
                        lhsT = at_flat[:, c0:c0 + 128]
                        rhs = bt_flat[:, c0:c0 + 128]
                        gg = gbank[g // 4][:, (g % 4) * 128:(g % 4 + 1) * 128]
                        # start=True lazily zeroes the WHOLE 2KB psum bank
                        # (zero region), so only the first group per bank may
                        # set it; groups 1-3 of the bank inherit pending-zero.
                        nc.tensor.matmul(
                            gg, lhsT, rhs,
                            start=(bm == 0 and t == 0 and g % 4 == 0),
                            stop=(bm == n_batch - 1 and t == kb - 1
                                  and g % 4 == 3))

            # ---- Phase 3a: stage G to SBUF, roundtrip via DRAM to gather ----
            # The psum->sbuf copy also regroups moving cols (l, r') -> (r', l)
            # so the per-row l-runs are contiguous for the gather DMA.
            # Per-bank pipeline: copy bank j, ship its slice to DRAM, gather
            # its 4 groups -- bank j+1's copy overlaps bank j's DMAs.
            gs = const.tile([128, 16 * 128], f32)
            for j in range(4):
                dstv = gs[:, j * 512:(j + 1) * 512].rearrange(
                    "p (g r l) -> p g r l", g=4, r=8)
                srcv = gbank[j][:].rearrange(
                    "p (g l r) -> p g r l", g=4, l=16)
                nc.scalar.activation(out=dstv, in_=srcv, func=Act.Copy,
                                     bias=0.0, scale=1.0)
                nc.sync.dma_start(gd_dram.ap()[:, j * 512:(j + 1) * 512],
                                  gs[:, j * 512:(j + 1) * 512])
                # gather: sc[g*8+r, 16h+l] = gd[8h+r, g*128 + 16r + l]
                # One DMA per g: dst [8 rows, 16h, 16l]; src diagonal AP,
                # r-stride = 2048 + 16 (flat DRAM elements), contiguous l.
                for g in range(4 * j, 4 * j + 4):
                    dst = sc[g * 8:(g + 1) * 8, :].rearrange(
                        "p (h l) -> p h l", h=16)
                    src = bass.AP(gd_dram, g * 128,
                                  [[2048 + 16, 8], [8 * 2048, 16], [1, 16]])
                    nc.sync.dma_start(dst, src)

        # ---- Phase 3b: staircase difference + first-index argmax ----
        # count[16h+l] = sc[16h+l] - sc[16(h+1)+l]
        nc.vector.tensor_tensor(out=dcnt[:, 0:240], in0=sc[:, 0:240],
                                in1=sc[:, 16:256], op=Alu.subtract)
        nc.vector.tensor_copy(dcnt[:, 240:256], sc[:, 240:256])
        # B l=8 slot was constant ones, so col 8 of dcnt currently holds
        # T_h - T_{h+1} (all q in [16h,16h+16)).  True count[16h+8]
        # = (T_h - T_{h+1}) - sum_{l != 8} count[16h+l] = 2*raw - rowsum.
        dc3 = dcnt[:].rearrange("p (h l) -> p h l", h=16)
        sall = const.tile([128, 16], f32)
        nc.vector.tensor_reduce(out=sall[:], in_=dc3,
                                axis=mybir.AxisListType.X, op=Alu.add)
        t16 = const.tile([128, 16, 1], f32)
        nc.vector.tensor_scalar(out=t16[:], in0=dc3[:, :, 8:9], scalar1=2.0,
                                scalar2=None, op0=Alu.mult)
        nc.vector.tensor_tensor(out=dc3[:, :, 8:9], in0=t16[:],
                                in1=sall[:].rearrange("p h -> p h ()"),
                                op=Alu.subtract)
        nc.vector.tensor_reduce(out=mx[:], in_=dcnt[:],
                                axis=mybir.AxisListType.X, op=Alu.max)
        t1 = const.tile([128, NBINS], f32)
        nc.vector.tensor_scalar(out=t1[:], in0=dcnt[:], scalar1=mx[:],
                                scalar2=1.0e6, op0=Alu.not_equal, op1=Alu.mult)
        nc.vector.tensor_tensor(out=t1[:], in0=t1[:], in1=iotat[:], op=Alu.add)
        nc.vector.tensor_reduce(out=dom[:], in_=t1[:],
                                axis=mybir.AxisListType.X, op=Alu.min)
        nc.vector.tensor_scalar(out=domp1[:], in0=dom[:], scalar1=1.0,
                                scalar2=None, op0=Alu.add)
        if dbg:
            nc.sync.dma_start(cnt_dram.ap(), dcnt[:])
            nc.sync.dma_start(dom_dram.ap(), dom[:])

        # ---- Phase 4: mask, convs, output ----
        # m = [dom <= x < dom+1]; att = 1 - m folded into (negated W1, b1)
        with ExitStack() as p4:
            px4 = p4.enter_context(tc.tile_pool(name="px4", bufs=4))
            pw = p4.enter_context(tc.tile_pool(name="pw", bufs=3))
            pz = p4.enter_context(tc.tile_pool(name="pz", bufs=3))
            pout = p4.enter_context(tc.tile_pool(name="pout", bufs=4))
            ps_z = p4.enter_context(tc.tile_pool(name="ps_z", bufs=3, space="PSUM"))
            ps_s = p4.enter_context(tc.tile_pool(name="ps_s", bufs=2, space="PSUM"))
            ps_b = p4.enter_context(tc.tile_pool(name="ps_b", bufs=3, space="PSUM"))

            for j in range(n_p4):
                sl = slice(j * p4_chunk, (j + 1) * p4_chunk)
                xt = px4.tile([P, p4_chunk], f32, tag="xt")
                nc.sync.dma_start(xt[:], x_dram.ap()[:, sl])
                ga = pw.tile([P, p4_chunk], f16, tag="ga")
                nc.vector.tensor_scalar(out=ga[:], in0=xt[:], scalar1=dom[:],
                                        scalar2=None, op0=Alu.is_ge)
                gb = pw.tile([P, p4_chunk], f16, tag="gb")
                nc.vector.tensor_scalar(out=gb[:], in0=xt[:], scalar1=domp1[:],
                                        scalar2=None, op0=Alu.is_ge)
                mt = pw.tile([P, p4_chunk], f16, tag="mt")
                nc.vector.tensor_tensor(out=mt[:], in0=ga[:], in1=gb[:],
                                        op=Alu.subtract)
                zt = pz.tile([P, p4_chunk], f16, tag="zt")
                st = pz.tile([2, p4_chunk], f16, tag="st")
                ot = pout.tile([P, p4_chunk], f32, tag="ot")
                for k in range(n_sub):
                    ssl = slice(k * 512, (k + 1) * 512)
                    zp = ps_z.tile([128, 512], f32, tag="zp")
                    nc.tensor.matmul(zp[:], w1hit[:], mt[:, ssl],
                                     start=True, stop=True)
                    nc.scalar.activation(out=zt[:, ssl], in_=zp[:], func=Act.Relu,
                                         bias=b1t[:], scale=1.0)
                    sp = ps_s.tile([2, 512], f32, tag="sp")
                    nc.tensor.matmul(sp[:], w2hit[:], zt[:, ssl],
                                     start=True, stop=True)
                    nc.scalar.activation(out=st[:, ssl], in_=sp[:], func=Act.Sigmoid,
                                         bias=b2t[:], scale=1.0)
                    bp = ps_b.tile([128, 512], f32, tag="bp")
                    nc.tensor.matmul(bp[:], selt[:], st[:, ssl], start=True, stop=True)
                    nc.vector.tensor_tensor(out=ot[:, ssl], in0=xt[:, ssl], in1=bp[:],
                                            op=Alu.mult)
                nc.sync.dma_start(out_dram.ap()[:, sl], ot[:])

    if not nc.is_finalized():
        nc.finalize()
    return nc


def _host_constants(conv1_w, conv1_b, bn1_gamma, bn1_beta, bn1_mean, bn1_var,
                    conv2_w, conv2_b, bn2_gamma, bn2_beta, bn2_mean, bn2_var):
    """Fold BN into conv weights (float64, cast f32) and build layout blocks.

    Phase 4 computes m = [dom <= x < dom+1] = 1 - att, so conv1 is applied with
    negated weights and bias shifted by the row sums: W1'(1-m) = (W1'*1 - W1'*m).
    """
    C = conv1_w.shape[0]
    inv1 = (bn1_gamma.astype(np.float64)
            / np.sqrt(bn1_var.astype(np.float64) + BN_EPS))
    w1f = conv1_w.astype(np.float64) * inv1[:, None]          # [o, c]
    b1f = (conv1_b.astype(np.float64) * inv1
           + bn1_beta.astype(np.float64)
           - bn1_mean.astype(np.float64) * inv1)              # [o]
    # att = 1 - m fold
    b1n = b1f + w1f.sum(axis=1)
    w1n = -w1f

    inv2 = (bn2_gamma.astype(np.float64)
            / np.sqrt(bn2_var.astype(np.float64) + BN_EPS))
    w2f = conv2_w[0].astype(np.float64) * inv2[0]             # [c]
    b2f = (conv2_b.astype(np.float64) * inv2
           + bn2_beta.astype(np.float64)
           - bn2_mean.astype(np.float64) * inv2)              # [1]

    fp16 = np.float16

    w1blk = np.zeros((128, 128), np.float32)
    w1t = w1n.T.astype(np.float32)                            # [c, o]
    w1blk[:C, :C] = w1t
    w1blk[C:, C:] = w1t
    w1hi = w1blk.astype(fp16)
    b1r = np.tile(b1n.astype(np.float32), 2).reshape(128, 1)

    w2blk = np.zeros((128, 2), np.float32)
    w2blk[:C, 0] = w2f.astype(np.float32)
    w2blk[C:, 1] = w2f.astype(np.float32)
    w2hi = w2blk.astype(fp16)
    b2r = np.full((2, 1), b2f[0], np.float32)

    sel2 = np.zeros((2, 128), fp16)
    sel2[0, :C] = 1.0
    sel2[1, C:] = 1.0

    iota = np.tile(np.arange(NBINS, dtype=np.float32), (128, 1))
    return dict(w1hi=w1hi, b1r=b1r, w2hi=w2hi, b2r=b2r, sel2=sel2, iota=iota)


def _run(x, conv1_w, conv1_b, bn1_gamma, bn1_beta, bn1_mean, bn1_var,
         conv2_w, conv2_b, bn2_gamma, bn2_beta, bn2_mean, bn2_var,
         trace=False):
    from concourse.bass_utils import run_bass_kernel_spmd

    x = np.asarray(x, np.float32)
    B, C, H, W = x.shape
    hw = H * W
    n_cores = 8
    bpc = B // n_cores  # batches per core

    key = ("nc", hw)
    if key not in _CACHE:
        _CACHE[key] = _build(hw=hw)
    nc = _CACHE[key]

    consts = _host_constants(
        np.asarray(conv1_w), np.asarray(conv1_b), np.asarray(bn1_gamma),
        np.asarray(bn1_beta), np.asarray(bn1_mean), np.asarray(bn1_var),
        np.asarray(conv2_w), np.asarray(conv2_b), np.asarray(bn2_gamma),
        np.asarray(bn2_beta), np.asarray(bn2_mean), np.asarray(bn2_var))

    xs = x.reshape(n_cores, bpc * C, hw)
    in_maps = [dict(x=np.ascontiguousarray(xs[i]), **consts) for i in range(n_cores)]

    res = run_bass_kernel_spmd(nc, in_maps, core_ids=list(range(n_cores)),
                               trace=trace)
    outs = [res.results[i]["out"].reshape(bpc, C, H, W) for i in range(n_cores)]
    return np.concatenate(outs, axis=0).astype(np.float32), res


def kernel(**inputs):
    out, _ = _run(**inputs)
    return out
